# revision 1
# baseline (speedup 1.0000x reference)
"""Trainium2 Bass kernel for nn_GPKANLayer (GP-KAN layer forward).

Math (reference):
    psi[b,o,i,m] = vk[o,i] * sqrt(l2/(l2+ex)) * exp(-0.5*(x[b,i]-z[o,i,m])^2/(l2+ex))
    em[b,o,i]   = sum_m psi * q_mu
    ev[b,o,i]   = sum_m psi^2 * (q_var + q_mu^2)
    out1[b,o]   = sum_i em
    out2[b,o]   = sum_i max(ev - em^2, EPS_EDGE)

Fast path (verified at runtime): z is identical across (o,i) and the
lengthscale is a single constant.  Then with D = l^2 + ex, a = 1/(2D):
    G[b,i,m]  = exp(-a*(x[b,i]-z[m])^2)          <- only large exp tensor (B*I*M)
    em[b,o,i] = sum_m G[b,i,m]  * W1[o,i,m],  W1 = vk*rho*q_mu
    ev[b,o,i] = sum_m G2[b,i,m] * W2[o,i,m],  W2 = (vk*rho)^2*(q_var+q_mu^2)
with G2 = G^2 = exp(-2a(x-z)^2) and rho = sqrt(l2/D).  The m-contraction
(and 4 i's at a time) runs on the tensor engine with block-diagonal
weights; per-i outputs stay separate for the max() clamp.

Sharding: batch dim across 8 cores, params replicated (folded on host).
"""

import numpy as np

B, O, I, M = 2048, 64, 64, 32
NCORES = 8
BLOC = B // NCORES          # 256 batch rows per core
IB = 4                      # i-values packed per matmul (K = IB*M = 128)
NIB = I // IB               # 16 i-blocks
NB = BLOC // 128            # 2 b-chunks of 128 per core
EPS_XVAR = 1e-06
EPS_QVAR = 1e-05
EPS_VAR = 1e-05
MIN_SCALE = 0.1
EPS_EDGE = 1e-06

_NC_CACHE = {}


def _build_nc(repeat=1):
    """Build + compile the per-core Bass program (SPMD, identical on all cores)."""
    import concourse.bass as bass
    import concourse.tile as tile
    from concourse import bacc, mybir

    f32 = mybir.dt.float32
    Exp = mybir.ActivationFunctionType.Exp

    nc = bacc.Bacc("TRN2", target_bir_lowering=False, debug=False)

    xT4_d = nc.dram_tensor("xT4", [128, NIB, BLOC], f32, kind="ExternalInput")
    w1_d = nc.dram_tensor("w1", [128, NIB, IB * O], f32, kind="ExternalInput")
    w2_d = nc.dram_tensor("w2", [128, NIB, IB * O], f32, kind="ExternalInput")
    zsc_d = nc.dram_tensor("zsc", [128, 1], f32, kind="ExternalInput")
    s1_d = nc.dram_tensor("s1c", [128, 1], f32, kind="ExternalInput")
    out1_d = nc.dram_tensor("out1", [BLOC, O], f32, kind="ExternalOutput")
    out2_d = nc.dram_tensor("out2", [BLOC, O], f32, kind="ExternalOutput")

    with tile.TileContext(nc) as tc:
        with (
            tc.tile_pool(name="const", bufs=1) as cpool,
            tc.tile_pool(name="work", bufs=3) as work,
            tc.tile_pool(name="post", bufs=4) as post,
            tc.tile_pool(name="psum", bufs=4, space="PSUM") as psum,
            tc.tile_pool(name="acc", bufs=1) as accp,
        ):
            xT4_t = cpool.tile([128, NIB, BLOC], f32, tag="xT4")
            w1_t = cpool.tile([128, NIB, IB * O], f32, tag="w1")
            w2_t = cpool.tile([128, NIB, IB * O], f32, tag="w2")
            zsc_t = cpool.tile([128, 1], f32, tag="zsc")
            s1_t = cpool.tile([128, 1], f32, tag="s1c")
            nc.sync.dma_start(xT4_t[:], xT4_d.ap()[:])
            nc.sync.dma_start(w1_t[:], w1_d.ap()[:])
            nc.sync.dma_start(w2_t[:], w2_d.ap()[:])
            nc.sync.dma_start(zsc_t[:], zsc_d.ap()[:])
            nc.sync.dma_start(s1_t[:], s1_d.ap()[:])

            acc1 = accp.tile([128, NB, O], f32, tag="acc1")
            acc2 = accp.tile([128, NB, O], f32, tag="acc2")

            def emit_body():
                nc.vector.memset(acc1[:], 0.0)
                nc.vector.memset(acc2[:], 0.0)
                for t in range(NIB):
                    # u = x/sqrt(2D) - z/sqrt(2D); s = u^2; G = exp(-s); G2 = exp(-2s)
                    u = work.tile([128, BLOC], f32, tag="u")
                    nc.vector.tensor_scalar(
                        u[:], xT4_t[:, t], s1_t[:, :1], zsc_t[:, :1],
                        op0=mybir.AluOpType.mult, op1=mybir.AluOpType.subtract,
                    )
                    s = work.tile([128, BLOC], f32, tag="s")
                    nc.vector.tensor_mul(s[:], u[:], u[:])
                    g = work.tile([128, BLOC], f32, tag="g")
                    nc.scalar.activation(g[:], s[:], Exp, scale=-1.0)
                    g2 = work.tile([128, BLOC], f32, tag="g2")
                    nc.scalar.activation(g2[:], s[:], Exp, scale=-2.0)

                    for bc in range(NB):
                        em = psum.tile([128, O, IB], f32, tag="em")
                        nc.tensor.matmul(
                            em[:], g[:, bass.ts(bc, 128)], w1_t[:, t],
                            start=True, stop=True,
                        )
                        ev = psum.tile([128, O, IB], f32, tag="ev")
                        nc.tensor.matmul(
                            ev[:], g2[:, bass.ts(bc, 128)], w2_t[:, t],
                            start=True, stop=True,
                        )
                        sq = post.tile([128, O, IB], f32, tag="sq")
                        nc.scalar.square(sq[:], em[:])
                        d = post.tile([128, O, IB], f32, tag="d")
                        nc.vector.tensor_sub(d[:], ev[:], sq[:])
                        dm = post.tile([128, O, IB], f32, tag="dm")
                        nc.vector.tensor_scalar_max(dm[:], d[:], EPS_EDGE)
                        r2 = post.tile([128, O], f32, tag="r2")
                        nc.vector.tensor_reduce(
                            r2[:], dm[:], axis=mybir.AxisListType.X,
                            op=mybir.AluOpType.add,
                        )
                        nc.vector.tensor_add(acc2[:, bc], acc2[:, bc], r2[:])
                        r1 = post.tile([128, O], f32, tag="r1")
                        nc.vector.tensor_reduce(
                            r1[:], em[:], axis=mybir.AxisListType.X,
                            op=mybir.AluOpType.add,
                        )
                        nc.vector.tensor_add(acc1[:, bc], acc1[:, bc], r1[:])

            if repeat == 1:
                emit_body()
            else:
                with tc.For_i(0, repeat, 1):
                    emit_body()

            for bc in range(NB):
                nc.sync.dma_start(out1_d.ap()[bass.ts(bc, 128), :], acc1[:, bc])
                nc.sync.dma_start(out2_d.ap()[bass.ts(bc, 128), :], acc2[:, bc])

    nc.compile()
    return nc


def _host_prep(x, zlin, lensq, w1d, w2d):
    """Per-core input maps for the fast path.  All float32 numpy."""
    f32 = np.float32
    D = f32(lensq) + f32(EPS_XVAR)
    s1 = (1.0 / np.sqrt(2.0 * D)).astype(f32)
    zsc = np.tile(zlin.astype(f32) * s1, IB).reshape(128, 1)
    s1c = np.full((128, 1), s1, f32)

    # w1d/w2d: [O, I, M] dense -> [128, NIB, IB*O] block-diagonal SBUF layout
    # k = i4*M + m ; n = o*IB + i4
    def blockify(wd):
        d2 = wd.reshape(O, NIB, IB, M).transpose(2, 3, 1, 0)  # [i4, m, t, o]
        out = np.zeros((IB, M, NIB, O, IB), f32)
        for i4 in range(IB):
            out[i4, :, :, :, i4] = d2[i4]
        return out.reshape(128, NIB, IB * O)

    w1h = blockify(w1d)
    w2h = blockify(w2d)

    in_maps = []
    for c in range(NCORES):
        xT = np.ascontiguousarray(x[c * BLOC:(c + 1) * BLOC].T.astype(f32))  # [I, BLOC]
        tmp = xT.reshape(NIB, IB, BLOC).transpose(1, 0, 2)  # [i4, t, b]
        xT4 = np.ascontiguousarray(
            np.broadcast_to(tmp[:, None], (IB, M, NIB, BLOC)).reshape(128, NIB, BLOC)
        )
        in_maps.append({"xT4": xT4, "w1": w1h, "w2": w2h, "zsc": zsc, "s1c": s1c})
    return in_maps


def _fallback(x, z, q_mu, q_log_var, log_scale, log_variance):
    """Generic numpy implementation (mirrors the reference exactly)."""
    x = np.asarray(x, np.float32)
    q_var = np.maximum(np.exp(np.asarray(q_log_var, np.float32)), EPS_QVAR)
    var_kern = np.maximum(np.exp(np.asarray(log_variance, np.float32)), EPS_VAR)
    lengthscale = np.maximum(np.exp(np.asarray(log_scale, np.float32)), MIN_SCALE)
    ell_sq = lengthscale ** 2
    denom = ell_sq + EPS_XVAR                      # [O, I]
    rho = np.sqrt(ell_sq / denom)
    z = np.asarray(z, np.float32)
    q_mu = np.asarray(q_mu, np.float32)
    w2 = q_var + q_mu ** 2
    o1 = np.empty((x.shape[0], O), np.float32)
    o2 = np.empty((x.shape[0], O), np.float32)
    for b0 in range(0, x.shape[0], 128):
        xs = x[b0:b0 + 128]
        diff = xs[:, None, :, None] - z[None]      # [b, O, I, M]
        psi = (var_kern * rho)[None, :, :, None] * np.exp(
            -0.5 * diff ** 2 / denom[None, :, :, None]
        )
        em = np.einsum("boim,oim->boi", psi, q_mu)
        ev = np.einsum("boim,oim->boi", psi ** 2, w2)
        o1[b0:b0 + 128] = em.sum(2)
        o2[b0:b0 + 128] = np.maximum(ev - em ** 2, EPS_EDGE).sum(2)
    return o1, o2


def _structure(x, z, q_mu, q_log_var, log_scale, log_variance):
    """Return (zlin, lensq) if the fast-path structure holds, else None."""
    if x.shape != (B, I) or z.shape != (O, I, M):
        return None
    z = np.asarray(z)
    if not (z == z[0, 0]).all():
        return None
    ls = np.maximum(np.exp(np.asarray(log_scale, np.float32)), np.float32(MIN_SCALE))
    if not (ls == ls.flat[0]).all():
        return None
    return np.asarray(z[0, 0], np.float32), np.float32(ls.flat[0]) ** 2


def kernel(x, z, q_mu, q_log_var, log_scale, log_variance):
    st = _structure(x, z, q_mu, q_log_var, log_scale, log_variance)
    if st is None:
        return _fallback(x, z, q_mu, q_log_var, log_scale, log_variance)
    zlin, lensq = st

    f32 = np.float32
    q_var = np.maximum(np.exp(np.asarray(q_log_var, f32)), f32(EPS_QVAR))
    vk = np.maximum(np.exp(np.asarray(log_variance, f32)), f32(EPS_VAR))
    D = lensq + f32(EPS_XVAR)
    rho = np.sqrt(lensq / D).astype(f32)
    c1 = (vk * rho).astype(f32)                       # [O, I]
    q_mu = np.asarray(q_mu, f32)
    w1d = c1[:, :, None] * q_mu                       # [O, I, M]
    w2d = (c1 ** 2)[:, :, None] * (q_var + q_mu ** 2)

    in_maps = _host_prep(np.asarray(x, f32), zlin, lensq, w1d, w2d)

    from concourse.bass_utils import run_bass_kernel_spmd

    if "nc" not in _NC_CACHE:
        _NC_CACHE["nc"] = _build_nc(repeat=1)
    nc = _NC_CACHE["nc"]
    res = run_bass_kernel_spmd(nc, in_maps, list(range(NCORES)))
    out1 = np.concatenate([res.results[c]["out1"] for c in range(NCORES)], 0)
    out2 = np.concatenate([res.results[c]["out2"] for c in range(NCORES)], 0)
    return out1, out2
